# revision 1
# baseline (speedup 1.0000x reference)
"""Trainium2 Bass kernel for a decoder block (self-attn + cross-attn + FFN).

Sharding: pure data-parallel over 8 shards = (batch b in 0..3, seq-half h in 0..1).
Each core processes 512 query tokens of one batch element. Keys are reordered
[own-half, other-half] so the SPMD program is identical on all cores; the causal
mask is per-core input data. No collectives.

On-chip layout convention:
  feature-major tile: [feature_part(128) x token_free]  (matmul inputs)
  token-major tile:   [token_part(128) x feature_free]  (softmax rows, LN, residual)

All matmuls run bf16 x bf16 -> fp32 PSUM. Residual/LN path stays fp32.
Softmax denominators come free from the attention O-matmul: the stationary
operand is a 2-block AP [V_head(64 cols) | ones(64 cols)], so PSUM rows 0-63
hold O_head and rows 64-127 hold the denominator replicated; one DVE divide
normalizes during PSUM evacuation.
"""

import os
import sys

for _p in ("/opt/trn_rl_repo",):
    if _p not in sys.path:
        sys.path.insert(0, _p)

import numpy as np
import ml_dtypes

import concourse.bass as bass
import concourse.tile as tile
from concourse import bacc, mybir
from concourse.bass import ts
from concourse.masks import make_identity

E = 1024          # model dim
T = 512           # query tokens per core
TC = 1024         # kv tokens
H = 16            # heads
S = 64            # head dim
HID = 4096        # ffn hidden
EPS = 1e-5
SCALE2 = float(E) ** -0.5   # e^-0.25 applied to q AND k == e^-0.5 on scores

BF16 = mybir.dt.bfloat16
F32 = mybir.dt.float32

ET = E // 128     # 8 feature tiles
TT = T // 128     # 4 query-token tiles
CT = TC // 128    # 8 key-token tiles
NCH = E // 512    # 2 psum-width chunks of the feature dim
HT = HID // 128   # 32 hidden tiles

WNAMES = ["sa_wq", "sa_wk", "sa_wv", "sa_wo", "ca_wq", "ca_wk", "ca_wv", "ca_wo"]


# V is stored interleaved per head: [128, CT, H, 128] where block h is
# [V_h (64 cols) | ones (64 cols)]. The O-matmul stationary operand is then a
# contiguous [128, 128] slice whose PSUM output rows 0-63 are O_h and rows
# 64-127 the softmax denominator replicated 64x (walrus requires a single
# free dim on the weights AP, so the ones columns must be interleaved).


def _attn_ln(nc, tc, name, qin, kvin, w_dram, mask_sb, resid_fn, xout_sb,
             xoutT_sb, id_f32, eps_sb, pools, preload=None, causal=False,
             q_dram=None, kv_load=None):
    """One attention block + residual + layernorm.

    qin(k)  -> [128, T] bf16 feature-major query-input tile k
    kvin(k) -> [128, TC] bf16 feature-major kv-input tile k
    w_dram  -> dict with wq, wk, wv, wo DRAM APs (natural [E, E] bf16)
    mask_sb -> [128, CT, 128] packed mask tile or None (causal only)
    resid_fn(t) -> [128, E] f32 token-major residual tile
    xout_sb -> [128, TT, E] f32 destination (post-LN, token-major)
    xoutT_sb-> [128, ET, T] bf16 destination (post-LN, feature-major) or None

    causal=True uses the interleaved query split: this core's query tile j
    holds global query block 2j+h, so key tile i is only needed for query
    tiles j >= i//2. Scores/exp/O are computed on the live suffix
    [128*(i//2):T] only, and the mask reduces to one 128x128 block per key
    tile at query column j = i//2 (triangular, all-dead pad, or all-zero,
    depending on the core's half -- that's per-core data, not program).
    """
    from contextlib import ExitStack

    with ExitStack() as st:
        wp = st.enter_context(tc.tile_pool(name=f"{name}_w", bufs=2))
        qp = st.enter_context(tc.tile_pool(name=f"{name}_q", bufs=1))
        kp = st.enter_context(tc.tile_pool(name=f"{name}_k", bufs=1))
        vp = st.enter_context(tc.tile_pool(name=f"{name}_v", bufs=1))
        ap_ = st.enter_context(tc.tile_pool(name=f"{name}_at", bufs=2))
        op = st.enter_context(tc.tile_pool(name=f"{name}_ot", bufs=1))
        xp = st.enter_context(tc.tile_pool(name=f"{name}_xr", bufs=2))
        sp = st.enter_context(tc.tile_pool(name=f"{name}_st", bufs=4))
        pp = st.enter_context(tc.tile_pool(name=f"{name}_ps", bufs=6, space="PSUM"))

        depth = 3 if causal else 2

        # ---- Q = (Xq @ Wq) * scale, feature-major [e_out, tq]
        def q_proj():
          wq_sb = wp.tile([128, ET, E], BF16, tag="w", name=f"{name}_wq")
          if q_dram is not None:
            # same byte size as an `at` tile: borrows one of its slots and
            # is released right after the Q projection. DMAs interleaved per
            # k-tile with wq so the first Q psum group starts almost
            # immediately (k-accumulation consumes tiles in k order).
            xqT_loc = ap_.tile([128, ET, T], BF16, tag="at", bufs=depth,
                               name=f"{name}_xqT")
            for k in range(ET):
                nc.sync.dma_start(out=xqT_loc[:, k, :],
                                  in_=q_dram[ts(k, 128), :])
                nc.sync.dma_start(out=wq_sb[:, k, :],
                                  in_=w_dram["wq"][ts(k, 128), :])
            qin_ = lambda k: xqT_loc[:, k, :]
          else:
            for m in range(ET):
                nc.sync.dma_start(out=wq_sb[:, m, :],
                                  in_=w_dram["wq"][ts(m, 128), :])
            qin_ = qin
          if preload is not None:
            preload()  # lower-priority input DMAs (mask, residual source)
          q_sb = qp.tile([128, ET, T], BF16, name=f"{name}_qsb")
          for m in range(ET):
            ps = pp.tile([128, T], F32, tag="mm", name=f"{name}_psq")
            for k in range(ET):
                nc.tensor.matmul(ps, lhsT=wq_sb[:, k, ts(m, 128)], rhs=qin_(k),
                                 start=(k == 0), stop=(k == ET - 1))
            nc.scalar.mul(q_sb[:, m, :], ps, SCALE2)
          return q_sb

        # ---- K feature-major [e_out, tk]
        def k_proj():
          wk_sb = wp.tile([128, ET, E], BF16, tag="w", name=f"{name}_wk")
          for m in range(ET):
            if kv_load is not None:
                kv_load(m)  # interleave kv-input tile m ahead of wk tile m
            nc.sync.dma_start(out=wk_sb[:, m, :], in_=w_dram["wk"][ts(m, 128), :])
          k_sb = kp.tile([128, ET, TC], BF16, name=f"{name}_ksb")
          for m in range(ET):
            for c in range(TC // 512):
                ps = pp.tile([128, 512], F32, tag="mm", name=f"{name}_psk")
                for k in range(ET):
                    nc.tensor.matmul(ps, lhsT=wk_sb[:, k, ts(m, 128)],
                                     rhs=kvin(k)[:, ts(c, 512)],
                                     start=(k == 0), stop=(k == ET - 1))
                nc.vector.tensor_copy(k_sb[:, m, ts(c, 512)], ps)
          return k_sb

        # ---- V token-major [tk, e] with 64 ones columns interleaved
        def v_proj():
          wv_sb = wp.tile([128, ET, E], BF16, tag="w", name=f"{name}_wv")
          for m in range(ET):
            nc.sync.dma_start(out=wv_sb[:, m, :], in_=w_dram["wv"][ts(m, 128), :])
          v_sb = vp.tile([128, CT, H, 128], BF16, name=f"{name}_vsb")
          for h in range(H):
            nc.vector.memset(v_sb[:, :, h, 64:128], 1.0)
          for t in range(CT):
            for c in range(NCH):
                ps = pp.tile([128, 512], F32, tag="mm", name=f"{name}_psv")
                for k in range(ET):
                    nc.tensor.matmul(ps, lhsT=kvin(k)[:, ts(t, 128)],
                                     rhs=wv_sb[:, k, ts(c, 512)],
                                     start=(k == 0), stop=(k == ET - 1))
                nc.vector.tensor_copy(
                    v_sb[:, t, 8 * c : 8 * c + 8, 0:64],
                    ps.rearrange("p (j s) -> p j s", j=8))
          return v_sb

        if causal:
            # time-to-first-matmul matters at kernel start: Q first
            q_sb = q_proj(); k_sb = k_proj(); v_sb = v_proj()
        else:
            # K/V depend only on the kv input, not the preceding layernorm:
            # emitting them first hides the LN/transpose chain of the
            # previous block under the K/V matmuls
            k_sb = k_proj(); v_sb = v_proj(); q_sb = q_proj()

        # ---- per-head: scores (transposed), exp, O with fused denominator.
        # Software-pipelined: head h's scores+exp are emitted before head
        # h-1's O-matmuls so PE has score work while ACT runs the exps.
        ot_sb = op.tile([128, ET, T], BF16, name=f"{name}_otsb")
        at_tiles = [None, None, None]

        def lo_of(i):
            return 128 * (i // 2) if causal else 0

        def scores(h):
            pm, po = 64 * (h % 2), h // 2
            at = ap_.tile([128, CT, T], BF16, tag="at", bufs=depth,
                          name=f"{name}_at")
            at_tiles[h % 3] = at
            for i in range(CT):
                lo = lo_of(i)
                ps = pp.tile([128, T], F32, tag="mm", name=f"{name}_pss")
                nc.tensor.matmul(ps[:, lo:T],
                                 lhsT=k_sb[pm : pm + 64, po, ts(i, 128)],
                                 rhs=q_sb[pm : pm + 64, po, lo:T],
                                 start=True, stop=True)
                if mask_sb is not None:
                    nc.vector.tensor_add(ps[:, lo : lo + 128],
                                         ps[:, lo : lo + 128],
                                         mask_sb[:, i, :])
                nc.scalar.activation(at[:, i, lo:T], ps[:, lo:T],
                                     func=mybir.ActivationFunctionType.Exp)

        def ovalue(h):
            pm, po = 64 * (h % 2), h // 2
            at = at_tiles[h % 3]
            ps_o = pp.tile([128, T], F32, tag="oo", bufs=2, name=f"{name}_pso")
            for i in range(CT):
                lo = lo_of(i)
                nc.tensor.matmul(ps_o[:, lo:T], lhsT=v_sb[:, i, h, :],
                                 rhs=at[:, i, lo:T],
                                 start=(i == 0), stop=(i == CT - 1))
            den = ap_.tile([64, T], F32, tag="den", name=f"{name}_den")
            nc.vector.tensor_copy(den, ps_o[64:128, :])
            nc.vector.reciprocal(den, den)
            nc.vector.tensor_mul(ot_sb[pm : pm + 64, po, :],
                                 ps_o[0:64, :], den)

        pd = depth - 1
        for h in range(pd):
            scores(h)
        for h in range(pd, H):
            scores(h)
            ovalue(h - pd)
        for h in range(H - pd, H):
            ovalue(h)

        # ---- output projection (token-major) + residual + LN
        wo_sb = wp.tile([128, ET, E], BF16, tag="w", name=f"{name}_wo")
        for m in range(ET):
            nc.sync.dma_start(out=wo_sb[:, m, :], in_=w_dram["wo"][ts(m, 128), :])
        for t in range(TT):
            xr = xp.tile([128, E], F32, tag="xr", name=f"{name}_xr")
            for c in range(NCH):
                ps = pp.tile([128, 512], F32, tag="mm", name=f"{name}_psw")
                for k in range(ET):
                    nc.tensor.matmul(ps, lhsT=ot_sb[:, k, ts(t, 128)],
                                     rhs=wo_sb[:, k, ts(c, 512)],
                                     start=(k == 0), stop=(k == ET - 1))
                nc.vector.tensor_add(xr[:, ts(c, 512)], ps,
                                     resid_fn(t)[:, ts(c, 512)])
            _ln(nc, tc, name, t, xr, xout_sb, sp, eps_sb)
            if xoutT_sb is not None:
                for m in range(ET):
                    # reuses the O-matmul psum slots (free during the LN phase)
                    pst = pp.tile([128, 128], F32, tag="oo", bufs=2,
                                  name=f"{name}_ptr")
                    nc.tensor.transpose(pst, xout_sb[:, t, ts(m, 128)], id_f32)
                    nc.scalar.copy(xoutT_sb[:, m, ts(t, 128)], pst)


def _ln(nc, tc, name, t, xr, xout_sb, sp, eps_sb):
    """LayerNorm of xr [128, E] f32 -> xout_sb[:, t, :]. gamma=1, beta=0
    (the reference's LN params are constants ones/zeros)."""
    stats = sp.tile([128, 2, 6], F32, tag="st", name=f"{name}_stats")
    for g in range(2):
        nc.vector.bn_stats(stats[:, g, :], xr[:, ts(g, 512)])
    mv = sp.tile([128, 2], F32, tag="mv", name=f"{name}_mv")
    nc.vector.bn_aggr(mv, stats)
    rstd = sp.tile([128, 1], F32, tag="rs", name=f"{name}_rstd")
    nc.scalar.activation(rstd, mv[:, 1:2],
                         func=mybir.ActivationFunctionType.Sqrt,
                         bias=eps_sb, scale=1.0)
    nc.vector.reciprocal(rstd, rstd)
    nc.vector.tensor_scalar(xout_sb[:, t, :], xr, mv[:, 0:1], rstd,
                            op0=mybir.AluOpType.subtract,
                            op1=mybir.AluOpType.mult)


def _emit(nc, tc, din, dout):
    from contextlib import ExitStack

    with ExitStack() as top:
        const = top.enter_context(tc.tile_pool(name="const", bufs=1))
        xtp = top.enter_context(tc.tile_pool(name="xt", bufs=1))
        mp = top.enter_context(tc.tile_pool(name="mask", bufs=1))
        rp = top.enter_context(tc.tile_pool(name="resid", bufs=2))
        rtp = top.enter_context(tc.tile_pool(name="residT", bufs=1))
        xqp = top.enter_context(tc.tile_pool(name="xq", bufs=2))
        outp = top.enter_context(tc.tile_pool(name="outp", bufs=2))

        id_f32 = const.tile([128, 128], F32, name="id_f32")
        make_identity(nc, id_f32)
        eps_sb = const.tile([128, 1], F32, name="eps_sb")
        nc.vector.memset(eps_sb, EPS)

        # persistent activations
        xkvT_sb = xtp.tile([128, ET, TC], BF16, tag="xt", name="xkvT_sb")
        mask_sb = mp.tile([128, CT, 128], BF16, name="mask_sb")
        x1_sb = rp.tile([128, TT, E], F32, tag="x", name="x1_sb")
        x1T_sb = rtp.tile([128, ET, T], BF16, tag="xT", name="x1T_sb")
        xq_tiles = [xqp.tile([128, E], F32, tag="xq", name=f"xq_{t}")
                    for t in range(TT)]

        def sa_kv_load(m):
            nc.sync.dma_start(out=xkvT_sb[:, m, :],
                              in_=din["xkvT"][ts(m, 128), :])

        def sa_preload():
            # emitted after the wq DMAs: these aren't needed until the
            # score and residual stages, so they shouldn't delay the first
            # matmuls
            for i in range(CT):
                nc.sync.dma_start(out=mask_sb[:, i, :],
                                  in_=din["maskT"][ts(i, 128), :])
            for t in range(TT):
                nc.sync.dma_start(out=xq_tiles[t], in_=din["xq"][ts(t, 128), :])

        _attn_ln(nc, tc, "sa",
                 qin=None, q_dram=din["xqT"],
                 kvin=lambda k: xkvT_sb[:, k, :],
                 w_dram={"wq": din["sa_wq"], "wk": din["sa_wk"],
                         "wv": din["sa_wv"], "wo": din["sa_wo"]},
                 mask_sb=mask_sb,
                 resid_fn=lambda t: xq_tiles[t],
                 xout_sb=x1_sb, xoutT_sb=x1T_sb,
                 id_f32=id_f32, eps_sb=eps_sb, pools=None,
                 preload=sa_preload, causal=True, kv_load=sa_kv_load)

        # cross-attention: kv from context
        ctxT_sb = xtp.tile([128, ET, TC], BF16, tag="xt", name="ctxT_sb")
        for m in range(ET):
            nc.sync.dma_start(out=ctxT_sb[:, m, :], in_=din["ctxT"][ts(m, 128), :])
        x2_sb = rp.tile([128, TT, E], F32, tag="x", name="x2_sb")
        x2T_sb = rtp.tile([128, ET, T], BF16, tag="xT", name="x2T_sb")

        _attn_ln(nc, tc, "ca",
                 qin=lambda k: x1T_sb[:, k, :],
                 kvin=lambda k: ctxT_sb[:, k, :],
                 w_dram={"wq": din["ca_wq"], "wk": din["ca_wk"],
                         "wv": din["ca_wv"], "wo": din["ca_wo"]},
                 mask_sb=None,
                 resid_fn=lambda t: x1_sb[:, t, :],
                 xout_sb=x2_sb, xoutT_sb=x2T_sb,
                 id_f32=id_f32, eps_sb=eps_sb, pools=None)

        # ---- FFN + residual + LN3 -> out
        with ExitStack() as st:
            wp = st.enter_context(tc.tile_pool(name="ffw", bufs=1))
            hp = st.enter_context(tc.tile_pool(name="ffh", bufs=1))
            xp = st.enter_context(tc.tile_pool(name="ffxr", bufs=2))
            sp = st.enter_context(tc.tile_pool(name="ffst", bufs=4))
            pp = st.enter_context(tc.tile_pool(name="ffps", bufs=4, space="PSUM"))

            # w1/w2 stream through two half-sized slots (tag fw, bufs=2):
            # w2's first half loads as soon as w1's first half is consumed,
            # overlapping the DMA with the remaining ffh matmuls.
            HH = HT // 2  # 16 hidden tiles per half
            ffh_sb = hp.tile([128, HT, T], BF16, name="ffh_sb")
            w2_halves = []
            for p_ in range(2):
                w1h = wp.tile([128, ET, HH * 128], BF16, tag="fw", bufs=2,
                              name=f"w1_sb{p_}")
                for m in range(ET):
                    nc.sync.dma_start(
                        out=w1h[:, m, :],
                        in_=din["ff_w1"][ts(m, 128), ts(p_, HH * 128)])
                for mm_ in range(HH):
                    m = p_ * HH + mm_
                    ps = pp.tile([128, T], F32, tag="mm", name="ffps1")
                    for k in range(ET):
                        nc.tensor.matmul(ps, lhsT=w1h[:, k, ts(mm_, 128)],
                                         rhs=x2T_sb[:, k, :],
                                         start=(k == 0), stop=(k == ET - 1))
                    nc.scalar.activation(ffh_sb[:, m, :], ps,
                                         func=mybir.ActivationFunctionType.Relu)
            for p_ in range(2):
                w2h = wp.tile([128, HH, E], BF16, tag="fw", bufs=2,
                              name=f"w2_sb{p_}")
                for mm_ in range(HH):
                    nc.sync.dma_start(out=w2h[:, mm_, :],
                                      in_=din["ff_w2"][ts(p_ * HH + mm_, 128), :])
                w2_halves.append(w2h)
            for t in range(TT):
                xr = xp.tile([128, E], F32, tag="xr", name="ff_xr")
                for c in range(NCH):
                    ps = pp.tile([128, 512], F32, tag="mm", name="ffps2")
                    for m in range(HT):
                        nc.tensor.matmul(ps, lhsT=ffh_sb[:, m, ts(t, 128)],
                                         rhs=w2_halves[m // HH][:, m % HH, ts(c, 512)],
                                         start=(m == 0), stop=(m == HT - 1))
                    nc.vector.tensor_add(xr[:, ts(c, 512)], ps,
                                         x2_sb[:, t, ts(c, 512)])
                out_t = outp.tile([128, E], F32, tag="out", name="out_t")
                _ln(nc, tc, "ff", 0, xr, out_t.rearrange("p (o e) -> p o e", o=1), sp, eps_sb)
                nc.sync.dma_start(out=dout[ts(t, 128), :], in_=out_t)


def build_program(n_iters=1):
    """n_iters>1 wraps the whole body in an on-device loop (benchmarking:
    amortizes the ~1.5ms per-dispatch RPC overhead of the axon path)."""
    nc = bacc.Bacc()
    din = {}

    def inp(name, shape, dt):
        din[name] = nc.dram_tensor(name, shape, dt, kind="ExternalInput").ap()

    inp("xq", [T, E], F32)
    inp("xqT", [E, T], BF16)
    inp("xkvT", [E, TC], BF16)
    inp("ctxT", [E, TC], BF16)
    inp("maskT", [TC, 128], BF16)
    for w in WNAMES:
        inp(w, [E, E], BF16)
    inp("ff_w1", [E, HID], BF16)
    inp("ff_w2", [HID, E], BF16)
    dout = nc.dram_tensor("out", [T, E], F32, kind="ExternalOutput").ap()

    with tile.TileContext(nc) as tc:
        if n_iters == 1:
            _emit(nc, tc, din, dout)
        else:
            with tc.For_i(0, n_iters, 1):
                _emit(nc, tc, din, dout)
    nc.compile()
    return nc


def own_rows(h):
    """Global token rows owned by seq-half h: interleaved 128-blocks
    {h, h+2, h+4, h+6} so the causal wavefront is balanced and key tile i
    is only needed by local query tiles j >= i//2."""
    return np.concatenate(
        [np.arange(128 * (2 * j + h), 128 * (2 * j + h) + 128) for j in range(TT)])


def shard_inputs(inputs):
    """Full inputs -> list of 8 per-core input maps."""
    bf = ml_dtypes.bfloat16
    x = np.asarray(inputs["x"], np.float32)
    ctx = np.asarray(inputs["context"], np.float32)
    wcast = {w: np.ascontiguousarray(np.asarray(inputs[w], np.float32).astype(bf))
             for w in WNAMES + ["ff_w1", "ff_w2"]}
    maps = []
    for c in range(8):
        b, h = divmod(c, 2)
        rows = own_rows(h)
        own = x[b, rows]                      # (T, E) own queries, token-major
        # packed mask: for key tile i the only query column that can need
        # masking is local block j = i//2 (global block g = 2j+h)
        maskP = np.zeros((TC, 128), np.float32)
        kk = np.arange(TC)
        for i in range(CT):
            g = 2 * (i // 2) + h
            kpos = 128 * i + np.arange(128)
            qpos = 128 * g + np.arange(128)
            maskP[128 * i : 128 * i + 128, :] = np.where(
                kpos[:, None] <= qpos[None, :], 0.0, -1e30)
        m = {
            "xq": np.ascontiguousarray(own),
            "xqT": np.ascontiguousarray(own.T.astype(bf)),
            "xkvT": np.ascontiguousarray(x[b].T.astype(bf)),
            "ctxT": np.ascontiguousarray(ctx[b].T.astype(bf)),
            "maskT": np.ascontiguousarray(maskP.astype(bf)),
        }
        m.update(wcast)
        maps.append(m)
    return maps


def gather_outputs(results):
    out = np.empty((4, 1024, E), np.float32)
    for c in range(8):
        b, h = divmod(c, 2)
        out[b, own_rows(h)] = results[c]["out"]
    return out


def kernel(**inputs):
    from concourse.bass_utils import run_bass_kernel_spmd

    nc = build_program()
    in_maps = shard_inputs(inputs)
    core_ids = list(range(8))
    res = run_bass_kernel_spmd(nc, in_maps, core_ids)
    return gather_outputs(res.results)


if __name__ == "__main__":
    nc = build_program()
    print("program built ok")

